# revision 9
# baseline (speedup 1.0000x reference)
"""Causal self-attention (B=4, T=2048, C=1024, H=16) on 8 TRN2 NeuronCores.

Sharding:
  - QKV + attention: tensor-parallel over heads (2 heads/core, all batches).
  - Output projection: data-parallel over tokens (1024 tokens/core),
    connected by one AllToAll per batch (1 MB/core each).

Layouts (everything feeds the PE in natural form, host pre-transposes):
  - host passes xT = x^T [C, B*T]; per-core W_attn q/k/v slices transposed
    [C, 128]; W_proj^T [C, C] replicated.
  - QKV computes qkvT [qkv_dim, tokens] (tokens moving, N=512, fp32r).
  - attention in S^T layout: S^T[s, t] = K @ Q^T per 128-key tile; softmax
    sums via a ones-column appended to PE-transposed V; exp on ScalarE with
    the 1/sqrt(D) folded into the activation scale; causal handled by
    trimming block ranges + one [128,128] additive tri-mask on the diagonal.
  - P@V in bf16 (P = exp output, V_aug = transposed V + ones col).
  - division by softmax sum via K=1 broadcast matmul + DVE multiply.
  - projection: lhsT = y^T tiles from the AllToAll, rhs = W_proj^T -> output
    lands token-major, DMA'd straight out.
"""

import sys

sys.path.insert(0, "/opt/trn_rl_repo")

import numpy as np

import concourse.bass as bass
import concourse.bacc as bacc
import concourse.mybir as mybir
import concourse.tile as tile
from concourse.bass_utils import run_bass_kernel_spmd

N_CORES = 8
B, T, C = 4, 2048, 1024
H, D = 16, 64
HPC = H // N_CORES          # heads per core = 2
BT = B * T                  # 8192 flattened tokens
QB = 512                    # query block
SB = 128                    # key tile
NQB = T // QB               # 4 query blocks per batch
NSB = T // SB               # 16 key tiles per batch
TOKS = BT // N_CORES        # 1024 output tokens per core
TPB = TOKS // B             # 256 tokens per (core, batch)

F32 = mybir.dt.float32
F32R = mybir.dt.float32r
BF16 = mybir.dt.bfloat16
EXP = mybir.ActivationFunctionType.Exp

# run_bass_kernel_spmd kwargs override (test.py sets {"trace": True})
RUN_KWARGS: dict = {}
LAST_RESULTS = None

_PROGRAM = None


def _build_program():
    nc = bacc.Bacc(num_devices=N_CORES)

    xT = nc.declare_dram_parameter("xT", [C, BT], F32, isOutput=False)
    wq = nc.declare_dram_parameter("wq", [C, 128], F32, isOutput=False)
    wk = nc.declare_dram_parameter("wk", [C, 128], F32, isOutput=False)
    wv = nc.declare_dram_parameter("wv", [C, 128], F32, isOutput=False)
    wp = nc.declare_dram_parameter("wp", [C, C], F32, isOutput=False)
    ntri = nc.declare_dram_parameter("ntri", [128, 128], F32, isOutput=False)
    ident = nc.declare_dram_parameter("ident", [64, 64], F32, isOutput=False)
    ones = nc.declare_dram_parameter("ones", [128, 1], F32, isOutput=False)
    one64 = nc.declare_dram_parameter("one64", [1, 64], F32, isOutput=False)
    out_ext = nc.declare_dram_parameter("out", [TOKS, C], F32, isOutput=True)

    # internal DRAM bounce buffers for the per-batch AllToAll
    sends = [nc.dram_tensor(f"send{b}", [N_CORES * 128, TPB], F32) for b in range(B)]
    recvs = [
        nc.dram_tensor(f"recv{b}", [N_CORES * 128, TPB], F32) for b in range(B)
    ]

    with tile.TileContext(nc) as tc:
        with (
            tc.tile_pool(name="const", bufs=1) as constp,
            tc.tile_pool(name="wgt", bufs=1) as wgtp,
            tc.tile_pool(name="qk", bufs=1) as qkp,
            tc.tile_pool(name="vt", bufs=4) as vtp,
            tc.tile_pool(name="vaug", bufs=4) as vaugp,
            tc.tile_pool(name="xt", bufs=9) as xtp,
            tc.tile_pool(name="pp", bufs=3) as ppool,
            tc.tile_pool(name="ysb", bufs=2) as ysbp,
            tc.tile_pool(name="osb", bufs=2) as osbp,
            tc.tile_pool(name="rv", bufs=10) as rvp,
            tc.tile_pool(name="sc", bufs=2) as scp,
            tc.tile_pool(name="qkvps", bufs=3, space="PSUM") as qkvps,
            tc.tile_pool(name="sps", bufs=2, space="PSUM") as sps,
            tc.tile_pool(name="yaps", bufs=2, space="PSUM") as yaps,
            tc.tile_pool(name="miscps", bufs=1, space="PSUM") as miscps,
        ):
            # ---------------- constants + weights ----------------
            ntri_s = constp.tile([128, 128], F32, tag="ntri")
            nc.sync.dma_start(out=ntri_s[:], in_=ntri[:])
            ident_s = constp.tile([64, 64], BF16, tag="ident")
            nc.gpsimd.dma_start(out=ident_s[:], in_=ident[:])
            ones_s = constp.tile([128, 1], F32, tag="ones")
            nc.sync.dma_start(out=ones_s[:], in_=ones[:])
            one64_s = constp.tile([1, 64], F32R, tag="one64")
            nc.gpsimd.dma_start(out=one64_s[:], in_=one64[:])

            # weights into SBUF as [128, n_ctiles * cols], chunk c = rows [128c:128c+128]
            wq_s = wgtp.tile([128, 8 * 128], F32R, tag="wq")
            wk_s = wgtp.tile([128, 8 * 128], F32R, tag="wk")
            wv_s = wgtp.tile([128, 8 * 128], F32R, tag="wv")
            for dst, src in ((wq_s, wq), (wk_s, wk), (wv_s, wv)):
                nc.gpsimd.dma_start(
                    out=dst[:].rearrange("p (c d) -> p c d", c=8),
                    in_=src[:].rearrange("(c p) d -> p c d", p=128),
                )
            wp_s = wgtp.tile([128, 8 * 1024], F32R, tag="wp")
            nc.gpsimd.dma_start(
                out=wp_s[:].rearrange("p (c d) -> p c d", c=8),
                in_=wp[:].rearrange("(c p) d -> p c d", p=128),
            )

            # full-sequence q^T / k^T, fp32r, 2 heads stacked on partitions
            qT = qkp.tile([128, BT], F32R, tag="qT")
            kT = qkp.tile([128, BT], F32R, tag="kT")

            vaug = {}  # (b, h) -> [128, NSB*65] bf16 tiles

            def phase1(b):
                """QKV for batch b + build V_aug tiles."""
                vt_h = [vtp.tile([64, T], BF16, tag="vt", name=f"vt{h}") for h in range(HPC)]
                for tb in range(T // QB):  # token blocks of 512
                    base = b * T + tb * QB
                    xts = []
                    for c in range(8):
                        xt_t = xtp.tile([128, QB], F32R, tag="xt")
                        nc.gpsimd.dma_start(
                            out=xt_t[:],
                            in_=xT[c * 128 : (c + 1) * 128, base : base + QB],
                        )
                        xts.append(xt_t)
                    pq = qkvps.tile([128, QB], F32, tag="qkvp")
                    pk = qkvps.tile([128, QB], F32, tag="qkvp")
                    pv = qkvps.tile([128, QB], F32, tag="qkvp")
                    for c in range(8):
                        rhs = xts[c][:]
                        st = dict(start=(c == 0), stop=(c == 7))
                        nc.tensor.matmul(
                            pq[:], wq_s[:, c * 128 : (c + 1) * 128], rhs, **st
                        )
                        nc.tensor.matmul(
                            pk[:], wk_s[:, c * 128 : (c + 1) * 128], rhs, **st
                        )
                        nc.tensor.matmul(
                            pv[:], wv_s[:, c * 128 : (c + 1) * 128], rhs, **st
                        )
                    nc.vector.tensor_copy(qT[:, base : base + QB], pq[:])
                    nc.vector.tensor_copy(kT[:, base : base + QB], pk[:])
                    tloc = tb * QB
                    for h in range(HPC):
                        nc.vector.tensor_copy(
                            vt_h[h][:, tloc : tloc + QB],
                            pv[h * 64 : (h + 1) * 64, :],
                        )
                # V_aug: PE-transpose each [64,128] slice of v^T to [128,64] + ones col
                for h in range(HPC):
                    va = vaugp.tile([128, NSB * 65], BF16, tag="vaug")
                    vaug[(b, h)] = va
                    for j in range(NSB):
                        tr = miscps.tile([128, 64], BF16, tag="misc")
                        nc.tensor.transpose(
                            tr[:], vt_h[h][:, j * SB : (j + 1) * SB], ident_s[:]
                        )
                        nc.vector.tensor_copy(va[:, j * 65 : j * 65 + 64], tr[:])
                        nc.vector.tensor_copy(
                            va[:, j * 65 + 64 : j * 65 + 65], ones_s[:]
                        )

            def phase2(b):
                """Attention for batch b; writes send buffer."""
                for i in range(NQB):
                    for h in range(HPC):
                        hlo, hhi = h * 64, (h + 1) * 64
                        va = vaug[(b, h)]
                        ya = yaps.tile([65, QB], F32, tag="yaug")
                        jmax = 4 * (i + 1)
                        pend = None  # (P tile, r, j) awaiting P@V
                        for j in range(jmax):
                            diag = j >= 4 * i
                            r = SB * j - QB * i if diag else 0
                            ncols = QB - r
                            sp = sps.tile([128, QB], F32, tag="sp")
                            nc.tensor.matmul(
                                sp[:, r:QB],
                                kT[hlo:hhi, b * T + j * SB : b * T + (j + 1) * SB],
                                qT[hlo:hhi, b * T + i * QB + r : b * T + (i + 1) * QB],
                                start=True,
                                stop=True,
                            )
                            if diag:
                                nc.vector.tensor_add(
                                    sp[:, r : r + 128], sp[:, r : r + 128], ntri_s[:]
                                )
                            pt = ppool.tile([128, QB], BF16, tag="P")
                            nc.scalar.activation(
                                pt[:, r:QB], sp[:, r:QB], EXP, scale=0.125
                            )
                            if pend is not None:
                                pp, pr, pj = pend
                                nc.tensor.matmul(
                                    ya[:, pr:QB],
                                    va[:, pj * 65 : pj * 65 + 65],
                                    pp[:, pr:QB],
                                    start=(pj == 0),
                                    stop=False,
                                )
                            pend = (pt, r, j)
                        pp, pr, pj = pend
                        nc.tensor.matmul(
                            ya[:, pr:QB],
                            va[:, pj * 65 : pj * 65 + 65],
                            pp[:, pr:QB],
                            start=(pj == 0),
                            stop=True,
                        )
                        # softmax division: recip of sums, gpsimd partition
                        # broadcast to SBUF, then one DVE multiply
                        rc = scp.tile([1, QB], F32, tag="recip")
                        nc.vector.reciprocal(rc[:], ya[64:65, :])
                        bc = scp.tile([64, QB], F32, tag="bcast")
                        nc.gpsimd.partition_broadcast(bc[:], rc[:])
                        yt = ysbp.tile([64, QB], F32, tag="ysb")
                        nc.vector.tensor_mul(yt[:], ya[0:64, :], bc[:])
                        # scatter the 512 queries into the 2 dest-core slices
                        for half in range(2):
                            m = 2 * i + half
                            nc.sync.dma_start(
                                out=sends[b][m * 128 + hlo : m * 128 + hhi, :],
                                in_=yt[:, half * TPB : (half + 1) * TPB],
                            )

            def a2a(b):
                nc.gpsimd.collective_compute(
                    "AllToAll",
                    mybir.AluOpType.bypass,
                    replica_groups=[list(range(N_CORES))],
                    ins=[sends[b][:]],
                    outs=[recvs[b][:]],
                )

            def phase3(b):
                """Projection for this core's 256 tokens of batch b."""
                rvs = []
                for c in range(8):
                    rt = rvp.tile([128, TPB], F32R, tag="rv")
                    nc.gpsimd.dma_start(
                        out=rt[:], in_=recvs[b][c * 128 : (c + 1) * 128, :]
                    )
                    rvs.append(rt)
                for tt in range(TPB // 128):
                    ob = osbp.tile([128, C], F32, tag="osb")
                    for co in range(2):
                        pj = miscps.tile([128, 512], F32, tag="misc")
                        for c in range(8):
                            nc.tensor.matmul(
                                pj[:],
                                rvs[c][:, tt * 128 : (tt + 1) * 128],
                                wp_s[:, c * 1024 + co * 512 : c * 1024 + (co + 1) * 512],
                                start=(c == 0),
                                stop=(c == 7),
                            )
                        nc.vector.tensor_copy(ob[:, co * 512 : (co + 1) * 512], pj[:])
                    nc.sync.dma_start(
                        out=out_ext[b * TPB + tt * 128 : b * TPB + (tt + 1) * 128, :],
                        in_=ob[:],
                    )

            # ---- emission order chosen so PE/ACT/collective overlap ----
            phase1(0)
            phase1(1)
            phase2(0)
            a2a(0)
            phase1(2)
            phase2(1)
            a2a(1)
            phase3(0)
            phase1(3)
            phase2(2)
            a2a(2)
            phase3(1)
            phase2(3)
            a2a(3)
            phase3(2)
            phase3(3)

    nc.finalize()
    return nc


def _prep_inputs(x, W_attn, b_attn, W_proj, b_proj):
    x = np.asarray(x, dtype=np.float32)
    W_attn = np.asarray(W_attn, dtype=np.float32)
    W_proj = np.asarray(W_proj, dtype=np.float32)

    xT = np.ascontiguousarray(x.reshape(BT, C).T)          # [C, BT]
    wpT = np.ascontiguousarray(W_proj.T)                   # [C, C]

    s = np.arange(128)[:, None]
    t = np.arange(128)[None, :]
    ntri = np.where(t >= s, 0.0, -1e9).astype(np.float32)  # valid: key <= query
    ident = np.eye(64, dtype=np.float32)
    ones = np.ones((128, 1), dtype=np.float32)
    one64 = np.ones((1, 64), dtype=np.float32)

    in_maps = []
    for k in range(N_CORES):
        r0 = k * HPC * D                                   # 128*k
        wq_k = np.ascontiguousarray(W_attn[r0 : r0 + 128, :].T)
        wk_k = np.ascontiguousarray(W_attn[C + r0 : C + r0 + 128, :].T)
        wv_k = np.ascontiguousarray(W_attn[2 * C + r0 : 2 * C + r0 + 128, :].T)
        in_maps.append(
            {
                "xT": xT,
                "wq": wq_k,
                "wk": wk_k,
                "wv": wv_k,
                "wp": wpT,
                "ntri": ntri,
                "ident": ident,
                "ones": ones,
                "one64": one64,
            }
        )
    return in_maps


def kernel(x, W_attn, b_attn, W_proj, b_proj):
    global _PROGRAM, LAST_RESULTS
    if _PROGRAM is None:
        _PROGRAM = _build_program()
    nc = _PROGRAM

    in_maps = _prep_inputs(x, W_attn, b_attn, W_proj, b_proj)
    res = run_bass_kernel_spmd(nc, in_maps, list(range(N_CORES)), **RUN_KWARGS)
    LAST_RESULTS = res

    out = np.empty((B, T, C), dtype=np.float32)
    for k in range(N_CORES):
        ok = res.results[k]["out"]                         # [TOKS, C]
        for b in range(B):
            out[b, k * TPB : (k + 1) * TPB, :] = ok[b * TPB : (b + 1) * TPB, :]
    return out
